# revision 3
# baseline (speedup 1.0000x reference)
"""Trainium2 Bass kernel: causal multi-head attention with RoPE (B=1, S=4096,
D=768, H=12) distributed over 8 NeuronCores.

Sharding strategy
-----------------
- Q rows are strided across cores (core c owns rows r = c mod 8).  Causal work
  is then uniform across cores, which is required because the SPMD program is
  identical on every core.
- K/V projections are computed on contiguous 512-row shards per core, RoPE'd
  and transposed locally, then AllGather'd (bf16) so every core holds full K/V.
- Attention runs in "transposed scores" layout: S^T[k, q] = K_rope @ Q_rope^T
  so that the AV matmul consumes exp(S^T) directly (no P transposes) and a
  ones-column appended to V yields the softmax denominators in the same
  accumulation.  Softmax is computed without max-subtraction (scores ~N(0,1)).
- RoPE pairs are de-interleaved by permuting W_q/W_k columns host-side so the
  rotation is a full-width unit-stride vector op.
"""

import math
import os
import sys

import numpy as np

sys.path.insert(0, "/opt/trn_rl_repo")

import ml_dtypes

import concourse.bass as bass
import concourse.mybir as mybir
import concourse.tile as tile
from concourse import bacc
from concourse.bass_utils import run_bass_kernel_spmd
from concourse.masks import make_identity

BF = ml_dtypes.bfloat16
F32 = mybir.dt.float32
BF16 = mybir.dt.bfloat16

S, D, H, DH = 4096, 768, 12, 64
NC = 8
SL = S // NC          # 512 rows per core (both q-strided and kv-contiguous)
NJ = SL // 128        # 4 q-tiles per head per core
NM = S // 128         # 32 k-tiles
NDC = D // 128        # 6 contraction chunks
EXPB = 3              # k-tiles per exp batch


def build_nc():
    nc = bacc.Bacc(None, target_bir_lowering=False, debug=False)

    xq_t = nc.dram_tensor("xq_t", [D, SL], BF16, kind="ExternalInput")
    xkv_t = nc.dram_tensor("xkv_t", [D, SL], BF16, kind="ExternalInput")
    wq = nc.dram_tensor("wq", [D, D], BF16, kind="ExternalInput")
    wk = nc.dram_tensor("wk", [D, D], BF16, kind="ExternalInput")
    wv = nc.dram_tensor("wv", [D, D], BF16, kind="ExternalInput")
    wo = nc.dram_tensor("wo", [D, D], BF16, kind="ExternalInput")
    cosq = nc.dram_tensor("cosq", [SL, H * 32], BF16, kind="ExternalInput")
    sinq = nc.dram_tensor("sinq", [SL, H * 32], BF16, kind="ExternalInput")
    cosk = nc.dram_tensor("cosk", [SL, H * 32], BF16, kind="ExternalInput")
    sink = nc.dram_tensor("sink", [SL, H * 32], BF16, kind="ExternalInput")
    mask8 = nc.dram_tensor("mask8", [128, 8 * 128], BF16, kind="ExternalInput")
    y_d = nc.dram_tensor("y", [SL, D], F32, kind="ExternalOutput")

    with tile.TileContext(nc) as tc:
        # ---- persistent pool (lives to the end) ----
        P1 = tc.alloc_tile_pool(name="persist", bufs=1)
        wo_sb = P1.tile([128, NDC, D], BF16)
        nc.sync.dma_start(out=wo_sb, in_=wo.rearrange("(c p) d -> p c d", p=128))
        mk_sb = P1.tile([128, 8, 128], BF16)
        nc.sync.dma_start(out=mk_sb, in_=mask8.rearrange("p (m q) -> p m q", m=8))
        ident = P1.tile([128, 128], BF16)
        make_identity(nc, ident)
        ones1 = P1.tile([1, DH], F32)
        nc.vector.memset(ones1, 1.0)
        qt_sb = P1.tile([128, NDC, SL], BF16)     # Q_rope^T, local
        att_sb = P1.tile([128, NDC, SL], BF16)    # attention out^T (normalized)
        ktg_sb = P1.tile([128, NDC, NM, 128], BF16)   # gathered K_rope^T
        vog_sb = P1.tile([128, NM, H, DH + 1], BF16)  # gathered V (+ones col)

        # DRAM bounce buffers for the collective
        PD = tc.alloc_tile_pool(name="dram", bufs=1, space="DRAM")
        kt_b = PD.tile([D, SL], BF16)
        v_b = PD.tile([SL, D], BF16)
        kt_g = PD.tile([NC * D, SL], BF16, addr_space="Shared")
        v_g = PD.tile([S, D], BF16, addr_space="Shared")

        # ---- projection + rope + transpose for one stream ----
        def proj_stream(x_t_sb, w_sb, cos_sb, sin_sb, dst_sb, v_w_sb, v_dst):
            """dst_sb: [128, NDC, SL] transposed rope'd output; optionally also
            compute the V projection (v_w_sb) into v_dst [128, NJ, D]."""
            PP = tc.alloc_tile_pool(name="proj_ps", bufs=2, space="PSUM")
            PT = tc.alloc_tile_pool(name="tr_ps", bufs=4, space="PSUM")
            PW = tc.alloc_tile_pool(name="proj_work", bufs=2)
            for st in range(NJ):
                n_ps = PP.tile([128, D], F32, tag="n_ps")
                for dc in range(NDC):
                    lt = x_t_sb[:, dc, st * 128:(st + 1) * 128]
                    nc.tensor.matmul(n_ps[:, 0:512], lt, w_sb[:, dc, 0:512],
                                     start=(dc == 0), stop=(dc == NDC - 1))
                    nc.tensor.matmul(n_ps[:, 512:768], lt, w_sb[:, dc, 512:768],
                                     start=(dc == 0), stop=(dc == NDC - 1))
                # rope in natural layout: per head [x0(32) | x1(32)]
                x0 = n_ps.rearrange("p (h d) -> p h d", h=H)[:, :, 0:32]
                x1 = n_ps.rearrange("p (h d) -> p h d", h=H)[:, :, 32:64]
                cs = cos_sb[:, st].rearrange("p (h d) -> p h d", h=H)
                sn = sin_sb[:, st].rearrange("p (h d) -> p h d", h=H)
                ta = PW.tile([128, H, 32], F32, tag="ta")
                tb = PW.tile([128, H, 32], F32, tag="tb")
                r_sb = PW.tile([128, H, 64], BF16, tag="r_sb")
                nc.vector.tensor_mul(ta, x0, cs)
                nc.vector.tensor_mul(tb, x1, sn)
                nc.vector.tensor_sub(r_sb[:, :, 0:32], ta, tb)
                nc.vector.tensor_mul(ta, x0, sn)
                nc.vector.tensor_mul(tb, x1, cs)
                nc.vector.tensor_add(r_sb[:, :, 32:64], ta, tb)
                # transpose to [dh, s]
                rf = r_sb.rearrange("p h d -> p (h d)")
                for dc in range(NDC):
                    t_ps = PT.tile([128, 128], BF16, tag="t_ps")
                    nc.tensor.transpose(t_ps, rf[:, dc * 128:(dc + 1) * 128], ident)
                    nc.vector.tensor_copy(
                        dst_sb[:, dc, st * 128:(st + 1) * 128], t_ps)
                if v_w_sb is not None:
                    v_ps = PP.tile([128, D], F32, tag="n_ps")
                    for dc in range(NDC):
                        lt = x_t_sb[:, dc, st * 128:(st + 1) * 128]
                        nc.tensor.matmul(v_ps[:, 0:512], lt, v_w_sb[:, dc, 0:512],
                                         start=(dc == 0), stop=(dc == NDC - 1))
                        nc.tensor.matmul(v_ps[:, 512:768], lt,
                                         v_w_sb[:, dc, 512:768],
                                         start=(dc == 0), stop=(dc == NDC - 1))
                    nc.scalar.activation(v_dst[:, st], v_ps,
                                         mybir.ActivationFunctionType.Copy)
            PW.release()
            PT.release()
            PP.release()

        # ---- K/V shard ----
        P2 = tc.alloc_tile_pool(name="kv_in", bufs=1)
        xkv_sb = P2.tile([128, NDC, SL], BF16)
        nc.sync.dma_start(out=xkv_sb, in_=xkv_t.rearrange("(c p) s -> p c s", p=128))
        wk_sb = P2.tile([128, NDC, D], BF16)
        nc.sync.dma_start(out=wk_sb, in_=wk.rearrange("(c p) d -> p c d", p=128))
        wv_sb = P2.tile([128, NDC, D], BF16)
        nc.sync.dma_start(out=wv_sb, in_=wv.rearrange("(c p) d -> p c d", p=128))
        ck_sb = P2.tile([128, NJ, H * 32], BF16)
        nc.sync.dma_start(out=ck_sb, in_=cosk.rearrange("(t p) d -> p t d", p=128))
        sk_sb = P2.tile([128, NJ, H * 32], BF16)
        nc.sync.dma_start(out=sk_sb, in_=sink.rearrange("(t p) d -> p t d", p=128))
        kts_sb = P2.tile([128, NDC, SL], BF16)
        vs_sb = P2.tile([128, NJ, D], BF16)

        proj_stream(xkv_sb, wk_sb, ck_sb, sk_sb, kts_sb, wv_sb, vs_sb)

        nc.sync.dma_start(out=kt_b.rearrange("(c p) s -> p c s", p=128), in_=kts_sb)
        nc.sync.dma_start(out=v_b.rearrange("(t p) d -> p t d", p=128), in_=vs_sb)
        nc.gpsimd.collective_compute(
            "AllGather", mybir.AluOpType.bypass,
            replica_groups=[list(range(NC))],
            ins=[kt_b[:]], outs=[kt_g[:]],
        )
        nc.gpsimd.collective_compute(
            "AllGather", mybir.AluOpType.bypass,
            replica_groups=[list(range(NC))],
            ins=[v_b[:]], outs=[v_g[:]],
        )

        # ---- Q shard (overlaps the collective) ----
        P3 = tc.alloc_tile_pool(name="q_in", bufs=1)
        xq_sb = P3.tile([128, NDC, SL], BF16)
        nc.sync.dma_start(out=xq_sb, in_=xq_t.rearrange("(c p) s -> p c s", p=128))
        wq_sb = P3.tile([128, NDC, D], BF16)
        nc.sync.dma_start(out=wq_sb, in_=wq.rearrange("(c p) d -> p c d", p=128))
        cq_sb = P3.tile([128, NJ, H * 32], BF16)
        nc.sync.dma_start(out=cq_sb, in_=cosq.rearrange("(t p) d -> p t d", p=128))
        sq_sb = P3.tile([128, NJ, H * 32], BF16)
        nc.sync.dma_start(out=sq_sb, in_=sinq.rearrange("(t p) d -> p t d", p=128))

        proj_stream(xq_sb, wq_sb, cq_sb, sq_sb, qt_sb, None, None)
        P3.release()
        P2.release()

        # ---- load gathered K/V into SBUF caches ----
        ktg_view = kt_g.rearrange("(r c p) s -> r c p s", r=NC, c=NDC)
        for dc in range(NDC):
            for r in range(NC):
                nc.sync.dma_start(
                    out=ktg_sb[:, dc, 4 * r:4 * (r + 1), :]
                        .rearrange("p m s -> p (m s)"),
                    in_=ktg_view[r, dc])
        vg_view = v_g.rearrange("(m p) (h d) -> m p h d", p=128, h=H)
        for m in range(NM):
            nc.sync.dma_start(out=vog_sb[:, m, :, 0:DH], in_=vg_view[m])
        nc.vector.memset(vog_sb[:, :, :, DH:DH + 1], 1.0)

        # ---- attention ----
        PS = tc.alloc_tile_pool(name="sc_ps", bufs=2, space="PSUM")
        PO = tc.alloc_tile_pool(name="o_ps", bufs=1, space="PSUM")
        PB = tc.alloc_tile_pool(name="b_ps", bufs=1, space="PSUM")
        PA = tc.alloc_tile_pool(name="att_work", bufs=3)

        for h in range(H):
            po = (h % 2) * 64
            dc = h // 2
            o_ps = PO.tile([DH + 1, SL], F32, tag="o_ps")
            for g in range(NJ):
                qoff = g * 128
                w = SL - qoff
                for b0 in range(8 * g, 8 * (g + 1), EXPB):
                    nb = min(EXPB, 8 * (g + 1) - b0)
                    sc_ps = PS.tile([128, EXPB, SL], F32, tag="sc_ps")
                    p_sb = PA.tile([128, EXPB, SL], BF16, tag="p_sb")
                    for mi in range(nb):
                        m = b0 + mi
                        nc.tensor.matmul(
                            sc_ps[:, mi, qoff:SL],
                            ktg_sb[po:po + DH, dc, m, :],
                            qt_sb[po:po + DH, dc, qoff:SL],
                            start=True, stop=True)
                    nc.scalar.activation(
                        p_sb[:, 0:nb, qoff:SL], sc_ps[:, 0:nb, qoff:SL],
                        mybir.ActivationFunctionType.Exp, scale=0.125)
                    ml0 = b0 - 8 * g
                    nc.vector.tensor_mul(
                        p_sb[:, 0:nb, qoff:qoff + 128],
                        p_sb[:, 0:nb, qoff:qoff + 128],
                        mk_sb[:, ml0:ml0 + nb, :])
                    for mi in range(nb):
                        m = b0 + mi
                        nc.tensor.matmul(
                            o_ps[:, qoff:SL],
                            vog_sb[:, m, h, :],
                            p_sb[:, mi, qoff:SL],
                            start=(m == 0), stop=(m == NM - 1))
            # normalize: att = o[0:64] * (1/denom) broadcast over partitions
            rd = PA.tile([1, SL], F32, tag="rd")
            nc.vector.reciprocal(rd, o_ps[DH:DH + 1, :])
            b_ps = PB.tile([DH, SL], F32, tag="b_ps")
            nc.tensor.matmul(b_ps, ones1, rd, start=True, stop=True)
            b_sb = PA.tile([DH, SL], F32, tag="b_sb")
            nc.vector.tensor_copy(b_sb, b_ps)
            nc.vector.tensor_mul(att_sb[po:po + DH, dc, :], o_ps[0:DH, :], b_sb)

        PA.release()
        PB.release()
        PO.release()
        PS.release()

        # ---- output projection ----
        PY = tc.alloc_tile_pool(name="y_ps", bufs=2, space="PSUM")
        PYW = tc.alloc_tile_pool(name="y_work", bufs=2)
        for j in range(NJ):
            y_ps = PY.tile([128, D], F32, tag="y_ps")
            for dc in range(NDC):
                lt = att_sb[:, dc, j * 128:(j + 1) * 128]
                nc.tensor.matmul(y_ps[:, 0:512], lt, wo_sb[:, dc, 0:512],
                                 start=(dc == 0), stop=(dc == NDC - 1))
                nc.tensor.matmul(y_ps[:, 512:768], lt, wo_sb[:, dc, 512:768],
                                 start=(dc == 0), stop=(dc == NDC - 1))
            y_sb = PYW.tile([128, D], F32, tag="y_sb")
            nc.vector.tensor_copy(y_sb, y_ps)
            nc.sync.dma_start(out=y_d[j * 128:(j + 1) * 128, :], in_=y_sb)
        PYW.release()
        PY.release()
        PD.release()
        P1.release()

    nc.compile()
    return nc


_NC_CACHE = None


def _get_nc():
    global _NC_CACHE
    if _NC_CACHE is None:
        _NC_CACHE = build_nc()
    return _NC_CACHE


def make_in_maps(x, rope_freqs, W_q, W_k, W_v, W_o):
    x2 = np.asarray(x, np.float32).reshape(S, D)
    cos = np.cos(np.asarray(rope_freqs, np.float32))
    sin = np.sin(np.asarray(rope_freqs, np.float32))
    perm = np.concatenate(
        [h * 64 + np.concatenate([np.arange(0, 64, 2), np.arange(1, 64, 2)])
         for h in range(H)])
    wq_p = np.asarray(W_q, np.float32)[:, perm].astype(BF)
    wk_p = np.asarray(W_k, np.float32)[:, perm].astype(BF)
    wv_b = np.asarray(W_v, np.float32).astype(BF)
    wo_b = np.asarray(W_o, np.float32).astype(BF)
    xT = np.ascontiguousarray(x2.T)

    in_maps = []
    for c in range(NC):
        xq_t = np.ascontiguousarray(xT[:, c::NC]).astype(BF)
        xkv_t = np.ascontiguousarray(xT[:, SL * c:SL * (c + 1)]).astype(BF)
        cq = np.ascontiguousarray(
            np.broadcast_to(cos[c::NC][:, None, :], (SL, H, 32))).reshape(SL, H * 32).astype(BF)
        sq = np.ascontiguousarray(
            np.broadcast_to(sin[c::NC][:, None, :], (SL, H, 32))).reshape(SL, H * 32).astype(BF)
        ck = np.ascontiguousarray(
            np.broadcast_to(cos[SL * c:SL * (c + 1)][:, None, :],
                            (SL, H, 32))).reshape(SL, H * 32).astype(BF)
        sk = np.ascontiguousarray(
            np.broadcast_to(sin[SL * c:SL * (c + 1)][:, None, :],
                            (SL, H, 32))).reshape(SL, H * 32).astype(BF)
        kr = np.arange(128)[:, None, None]
        ml = np.arange(8)[None, :, None]
        col = np.arange(128)[None, None, :]
        mk = (128 * ml + kr <= 8 * col + c).astype(BF).reshape(128, 8 * 128)
        in_maps.append({
            "xq_t": xq_t, "xkv_t": xkv_t,
            "wq": wq_p, "wk": wk_p, "wv": wv_b, "wo": wo_b,
            "cosq": cq, "sinq": sq, "cosk": ck, "sink": sk,
            "mask8": mk,
        })
    return in_maps


def kernel(x, rope_freqs, W_q, W_k, W_v, W_o):
    nc = _get_nc()
    in_maps = make_in_maps(x, rope_freqs, W_q, W_k, W_v, W_o)
    res = run_bass_kernel_spmd(nc, in_maps, core_ids=list(range(NC)))
    out = np.empty((S, D), np.float32)
    for c in range(NC):
        out[c::NC, :] = res.results[c]["y"]
    return out.reshape(1, S, D)


# revision 14
# speedup vs baseline: 1.0379x; 1.0379x over previous
"""Trainium2 Bass kernel: causal multi-head attention with RoPE (B=1, S=4096,
D=768, H=12) distributed over 8 NeuronCores.

Sharding strategy
-----------------
- Q rows are strided across cores (core c owns rows r = c mod 8).  Causal work
  is then uniform across cores, which is required because the SPMD program is
  identical on every core.
- K/V projections are computed on contiguous 512-row shards per core, RoPE'd
  and transposed locally, then AllGather'd (bf16) so every core holds full K/V.
- Attention runs in "transposed scores" layout: S^T[k, q] = K_rope @ Q_rope^T
  so that the AV matmul consumes exp(S^T) directly (no P transposes) and a
  ones-column appended to V yields the softmax denominators in the same
  accumulation.  Softmax is computed without max-subtraction (scores ~N(0,1)).
- RoPE pairs are de-interleaved by permuting W_q/W_k columns host-side so the
  rotation is a full-width unit-stride vector op.
"""

import math
import os
import sys

import numpy as np

sys.path.insert(0, "/opt/trn_rl_repo")

import ml_dtypes

import concourse.bass as bass
import concourse.mybir as mybir
import concourse.tile as tile
from concourse import bacc
from concourse.bass_utils import run_bass_kernel_spmd
from concourse.masks import make_identity

BF = ml_dtypes.bfloat16
F32 = mybir.dt.float32
BF16 = mybir.dt.bfloat16

S, D, H, DH = 4096, 768, 12, 64
NC = 8
SL = S // NC          # 512 rows per core (both q-strided and kv-contiguous)
NJ = SL // 128        # 4 q-tiles per head per core
NM = S // 128         # 32 k-tiles
NDC = D // 128        # 6 contraction chunks
EXPB = 3              # k-tiles per exp batch
DMAT = True


def build_nc():
    nc = bacc.Bacc(None, target_bir_lowering=False, debug=False)

    xq_t = nc.dram_tensor("xq_t", [D, SL], BF16, kind="ExternalInput")
    xkv_t = nc.dram_tensor("xkv_t", [D, SL], BF16, kind="ExternalInput")
    wq = nc.dram_tensor("wq", [D, D], BF16, kind="ExternalInput")
    wk = nc.dram_tensor("wk", [D, D], BF16, kind="ExternalInput")
    wv = nc.dram_tensor("wv", [D, D], BF16, kind="ExternalInput")
    wo = nc.dram_tensor("wo", [D, D], BF16, kind="ExternalInput")
    cosq = nc.dram_tensor("cosq", [SL, H * 32], BF16, kind="ExternalInput")
    sinq = nc.dram_tensor("sinq", [SL, H * 32], BF16, kind="ExternalInput")
    cosk = nc.dram_tensor("cosk", [SL, H * 32], BF16, kind="ExternalInput")
    sink = nc.dram_tensor("sink", [SL, H * 32], BF16, kind="ExternalInput")
    mask8 = nc.dram_tensor("mask8", [128, 8 * 128], BF16, kind="ExternalInput")
    y_d = nc.dram_tensor("y", [SL, D], F32, kind="ExternalOutput")

    with tile.TileContext(nc) as tc:
        # ---- persistent pool (lives to the end) ----
        P1 = tc.alloc_tile_pool(name="persist", bufs=1)
        wo_sb = P1.tile([128, NDC, D], BF16)
        mk_sb = P1.tile([128, 8, 128], BF16)
        ident = P1.tile([128, 128], BF16)
        make_identity(nc, ident)
        qt_sb = P1.tile([128, NDC, SL], BF16)     # Q_rope^T, local
        att_sb = P1.tile([128, NDC, SL], BF16)    # attention out^T (normalized)
        ktg_sb = P1.tile([128, NDC, NM, 128], BF16)   # gathered K_rope^T
        vog_sb = P1.tile([128, NM, H, DH + 1], BF16)  # gathered V (+ones col)

        KT_N = D * SL
        PD = tc.alloc_tile_pool(name="dram", bufs=1, space="DRAM")
        kt_b = PD.tile([KT_N], BF16)
        v_b = PD.tile([KT_N], BF16)
        kt_g = PD.tile([NC * KT_N], BF16, addr_space="Shared")
        v_g = PD.tile([NC * KT_N], BF16, addr_space="Shared")

        # ---- projection + rope + transpose for one stream ----
        def proj_stream(x_t_sb, w_sb, cos_sb, sin_sb, dst_sb):
            """dst_sb: [128, NDC, SL] transposed rope'd projection."""
            PP = tc.alloc_tile_pool(name="proj_ps", bufs=2, space="PSUM")
            PT = tc.alloc_tile_pool(name="tr_ps", bufs=4, space="PSUM")
            PW = tc.alloc_tile_pool(name="proj_work", bufs=2)
            for st in range(NJ):
                n_ps = PP.tile([128, D], F32, tag="n_ps")
                for dc in range(NDC):
                    lt = x_t_sb[:, dc, st * 128:(st + 1) * 128]
                    nc.tensor.matmul(n_ps[:, 0:512], lt, w_sb[:, dc, 0:512],
                                     start=(dc == 0), stop=(dc == NDC - 1))
                    nc.tensor.matmul(n_ps[:, 512:768], lt, w_sb[:, dc, 512:768],
                                     start=(dc == 0), stop=(dc == NDC - 1))
                # rope in natural layout: per head [x0(32) | x1(32)]
                x0 = n_ps.rearrange("p (h d) -> p h d", h=H)[:, :, 0:32]
                x1 = n_ps.rearrange("p (h d) -> p h d", h=H)[:, :, 32:64]
                cs = cos_sb[:, st].rearrange("p (h d) -> p h d", h=H)
                sn = sin_sb[:, st].rearrange("p (h d) -> p h d", h=H)
                ta = PW.tile([128, H, 32], F32, tag="ta")
                tb = PW.tile([128, H, 32], F32, tag="tb")
                r_sb = PW.tile([128, H, 64], BF16, tag="r_sb")
                nc.vector.tensor_mul(ta, x0, cs)
                nc.vector.tensor_mul(tb, x1, sn)
                nc.vector.tensor_sub(r_sb[:, :, 0:32], ta, tb)
                nc.vector.tensor_mul(ta, x0, sn)
                nc.vector.tensor_mul(tb, x1, cs)
                nc.vector.tensor_add(r_sb[:, :, 32:64], ta, tb)
                # transpose to [dh, s]
                rf = r_sb.rearrange("p h d -> p (h d)")
                for dc in range(NDC):
                    if DMAT:
                        nc.sync.dma_start(
                            out=dst_sb[:, dc, st * 128:(st + 1) * 128],
                            in_=rf[:, dc * 128:(dc + 1) * 128], transpose=True)
                    else:
                        t_ps = PT.tile([128, 128], BF16, tag="t_ps")
                        nc.tensor.transpose(
                            t_ps, rf[:, dc * 128:(dc + 1) * 128], ident)
                        nc.vector.tensor_copy(
                            dst_sb[:, dc, st * 128:(st + 1) * 128], t_ps)
            PW.release()
            PT.release()
            PP.release()

        def v_proj(x_t_sb, v_w_sb, v_dst):
            PP = tc.alloc_tile_pool(name="vproj_ps", bufs=2, space="PSUM")
            for st in range(NJ):
                v_ps = PP.tile([128, D], F32, tag="v_ps")
                for dc in range(NDC):
                    lt = x_t_sb[:, dc, st * 128:(st + 1) * 128]
                    nc.tensor.matmul(v_ps[:, 0:512], lt, v_w_sb[:, dc, 0:512],
                                     start=(dc == 0), stop=(dc == NDC - 1))
                    nc.tensor.matmul(v_ps[:, 512:768], lt,
                                     v_w_sb[:, dc, 512:768],
                                     start=(dc == 0), stop=(dc == NDC - 1))
                nc.vector.tensor_copy(v_dst[:, st], v_ps)
            PP.release()

        # ---- K/V shard ----
        P2 = tc.alloc_tile_pool(name="kv_in", bufs=1)
        xkv_sb = P2.tile([128, NDC, SL], BF16)
        nc.sync.dma_start(out=xkv_sb, in_=xkv_t.rearrange("(c p) s -> p c s", p=128))
        wk_sb = P2.tile([128, NDC, D], BF16)
        nc.sync.dma_start(out=wk_sb, in_=wk.rearrange("(c p) d -> p c d", p=128))
        wv_sb = P2.tile([128, NDC, D], BF16)
        nc.sync.dma_start(out=wv_sb, in_=wv.rearrange("(c p) d -> p c d", p=128))
        ck_sb = P2.tile([128, NJ, H * 32], BF16)
        nc.sync.dma_start(out=ck_sb, in_=cosk.rearrange("(t p) d -> p t d", p=128))
        sk_sb = P2.tile([128, NJ, H * 32], BF16)
        nc.sync.dma_start(out=sk_sb, in_=sink.rearrange("(t p) d -> p t d", p=128))
        kts_sb = P2.tile([128, NDC, SL], BF16)
        vs_sb = P2.tile([128, NJ, D], BF16)

        proj_stream(xkv_sb, wk_sb, ck_sb, sk_sb, kts_sb)
        nc.sync.dma_start(
            out=kt_b[:].rearrange("(c p s) -> p c s", p=128, c=NDC),
            in_=kts_sb)
        nc.gpsimd.collective_compute(
            "AllGather", mybir.AluOpType.bypass,
            replica_groups=[list(range(NC))],
            ins=[kt_b[:]], outs=[kt_g[:]],
        )
        v_proj(xkv_sb, wv_sb, vs_sb)
        nc.sync.dma_start(
            out=v_b[:].rearrange("(t p d) -> p t d", p=128, t=NJ),
            in_=vs_sb)
        nc.gpsimd.collective_compute(
            "AllGather", mybir.AluOpType.bypass,
            replica_groups=[list(range(NC))],
            ins=[v_b[:]], outs=[v_g[:]],
        )

        # ---- Q shard (overlaps the collective) ----
        P3 = tc.alloc_tile_pool(name="q_in", bufs=1)
        xq_sb = P3.tile([128, NDC, SL], BF16)
        nc.sync.dma_start(out=xq_sb, in_=xq_t.rearrange("(c p) s -> p c s", p=128))
        wq_sb = P3.tile([128, NDC, D], BF16)
        nc.sync.dma_start(out=wq_sb, in_=wq.rearrange("(c p) d -> p c d", p=128))
        cq_sb = P3.tile([128, NJ, H * 32], BF16)
        nc.sync.dma_start(out=cq_sb, in_=cosq.rearrange("(t p) d -> p t d", p=128))
        sq_sb = P3.tile([128, NJ, H * 32], BF16)
        nc.sync.dma_start(out=sq_sb, in_=sinq.rearrange("(t p) d -> p t d", p=128))

        proj_stream(xq_sb, wq_sb, cq_sb, sq_sb, qt_sb)
        nc.sync.dma_start(out=wo_sb, in_=wo.rearrange("(c p) d -> p c d", p=128))
        nc.sync.dma_start(out=mk_sb, in_=mask8.rearrange("p (m q) -> p m q", m=8))
        P3.release()
        P2.release()

        # ---- load gathered K/V into SBUF caches ----
        nc.vector.memset(vog_sb[:, :, :, DH:DH + 1], 1.0)
        ktg_view = kt_g.rearrange("(r c p s) -> r c p s", r=NC, c=NDC, p=128)
        for r in range(NC):
            for dc in range(NDC):
                nc.sync.dma_start(
                    out=ktg_sb[:, dc, 4 * r:4 * (r + 1), :]
                        .rearrange("p m s -> p (m s)"),
                    in_=ktg_view[r, dc])
        vg_view = v_g.rearrange("(r t p h d) -> r t p h d", r=NC, t=NJ, p=128, h=H)
        for r in range(NC):
            for t in range(NJ):
                nc.gpsimd.dma_start(
                    out=vog_sb[:, 4 * r + t, :, 0:DH], in_=vg_view[r, t])

        # ---- attention ----
        PS = tc.alloc_tile_pool(name="sc_ps", bufs=2, space="PSUM")
        PO = tc.alloc_tile_pool(name="o_ps", bufs=2, space="PSUM")
        PA = tc.alloc_tile_pool(name="att_work", bufs=8)
        PRD = tc.alloc_tile_pool(name="rd_dram", bufs=2, space="DRAM")

        for h in range(H):
            po = (h % 2) * 64
            dc = h // 2
            o_ps = PO.tile([DH + 1, SL], F32, tag="o_ps")
            for g in range(NJ):
                qoff = g * 128
                ml0 = 0
                while ml0 < 8:
                    # narrow the window: columns left of 16*ml0 are fully
                    # masked for every k-tile in this batch
                    woff = qoff + 16 * ml0
                    wb = SL - woff
                    # slot stride: each member must stay inside one PSUM bank
                    slot = 512 if wb > 256 else (256 if wb > 128 else 128)
                    nb = min(8 - ml0, 1536 // slot)
                    mw = 128 - 16 * ml0
                    sc_ps = PS.tile([128, 1536], F32, tag="sc_ps")
                    scv = sc_ps.rearrange(
                        "p (m q) -> p m q", q=slot)[:, 0:nb, 0:wb]
                    p_sb = PA.tile([128, 1536], BF16, tag="p_sb")
                    pv = p_sb[:, 0:nb * wb].rearrange("p (m q) -> p m q", m=nb)
                    for mi in range(nb):
                        m = 8 * g + ml0 + mi
                        nc.tensor.matmul(
                            scv[:, mi, :],
                            ktg_sb[po:po + DH, dc, m, :],
                            qt_sb[po:po + DH, dc, woff:SL],
                            start=True, stop=True)
                    nc.scalar.activation(
                        pv, scv, mybir.ActivationFunctionType.Exp, scale=0.125)
                    nc.vector.tensor_mul(
                        pv[:, :, 0:mw], pv[:, :, 0:mw],
                        mk_sb[:, ml0:ml0 + nb, 16 * ml0:128])
                    for mi in range(nb):
                        m = 8 * g + ml0 + mi
                        nc.tensor.matmul(
                            o_ps[:, woff:SL],
                            vog_sb[:, m, h, :],
                            pv[:, mi, :],
                            start=(m == 0), stop=(m == NM - 1))
                    ml0 += nb
            # normalize: att = o[0:64] * (1/denom) broadcast over partitions
            rd = PA.tile([1, SL], F32, tag="rd")
            nc.vector.reciprocal(rd, o_ps[DH:DH + 1, :])
            rd_d = PRD.tile([SL], F32, tag="rd_d")
            nc.sync.dma_start(out=rd_d[None, :], in_=rd)
            b_sb = PA.tile([DH, SL], F32, tag="b_sb")
            nc.sync.dma_start(
                out=b_sb,
                in_=bass.AP(tensor=rd_d.tensor, offset=rd_d.offset,
                            ap=[[0, DH], [1, SL]]))
            nc.vector.tensor_mul(att_sb[po:po + DH, dc, :], o_ps[0:DH, :], b_sb)

        PRD.release()
        PA.release()
        PO.release()
        PS.release()

        # ---- output projection ----
        PY = tc.alloc_tile_pool(name="y_ps", bufs=2, space="PSUM")
        PYW = tc.alloc_tile_pool(name="y_work", bufs=2)
        for j in range(NJ):
            y_ps = PY.tile([128, D], F32, tag="y_ps")
            for dc in range(NDC):
                lt = att_sb[:, dc, j * 128:(j + 1) * 128]
                nc.tensor.matmul(y_ps[:, 0:512], lt, wo_sb[:, dc, 0:512],
                                 start=(dc == 0), stop=(dc == NDC - 1))
                nc.tensor.matmul(y_ps[:, 512:768], lt, wo_sb[:, dc, 512:768],
                                 start=(dc == 0), stop=(dc == NDC - 1))
            y_sb = PYW.tile([128, D], F32, tag="y_sb")
            nc.vector.tensor_copy(y_sb, y_ps)
            nc.sync.dma_start(out=y_d[j * 128:(j + 1) * 128, :], in_=y_sb)
        PYW.release()
        PY.release()
        PD.release()
        P1.release()

    nc.compile()
    return nc


_NC_CACHE = None


def _get_nc():
    global _NC_CACHE
    if _NC_CACHE is None:
        _NC_CACHE = build_nc()
    return _NC_CACHE


def make_in_maps(x, rope_freqs, W_q, W_k, W_v, W_o):
    x2 = np.asarray(x, np.float32).reshape(S, D)
    cos = np.cos(np.asarray(rope_freqs, np.float32))
    sin = np.sin(np.asarray(rope_freqs, np.float32))
    perm = np.concatenate(
        [h * 64 + np.concatenate([np.arange(0, 64, 2), np.arange(1, 64, 2)])
         for h in range(H)])
    wq_p = np.asarray(W_q, np.float32)[:, perm].astype(BF)
    wk_p = np.asarray(W_k, np.float32)[:, perm].astype(BF)
    wv_b = np.asarray(W_v, np.float32).astype(BF)
    wo_b = np.asarray(W_o, np.float32).astype(BF)
    xT = np.ascontiguousarray(x2.T)

    in_maps = []
    for c in range(NC):
        xq_t = np.ascontiguousarray(xT[:, c::NC]).astype(BF)
        xkv_t = np.ascontiguousarray(xT[:, SL * c:SL * (c + 1)]).astype(BF)
        cq = np.ascontiguousarray(
            np.broadcast_to(cos[c::NC][:, None, :], (SL, H, 32))).reshape(SL, H * 32).astype(BF)
        sq = np.ascontiguousarray(
            np.broadcast_to(sin[c::NC][:, None, :], (SL, H, 32))).reshape(SL, H * 32).astype(BF)
        ck = np.ascontiguousarray(
            np.broadcast_to(cos[SL * c:SL * (c + 1)][:, None, :],
                            (SL, H, 32))).reshape(SL, H * 32).astype(BF)
        sk = np.ascontiguousarray(
            np.broadcast_to(sin[SL * c:SL * (c + 1)][:, None, :],
                            (SL, H, 32))).reshape(SL, H * 32).astype(BF)
        kr = np.arange(128)[:, None, None]
        ml = np.arange(8)[None, :, None]
        col = np.arange(128)[None, None, :]
        mk = (128 * ml + kr <= 8 * col + c).astype(BF).reshape(128, 8 * 128)
        in_maps.append({
            "xq_t": xq_t, "xkv_t": xkv_t,
            "wq": wq_p, "wk": wk_p, "wv": wv_b, "wo": wo_b,
            "cosq": cq, "sinq": sq, "cosk": ck, "sink": sk,
            "mask8": mk,
        })
    return in_maps


def kernel(x, rope_freqs, W_q, W_k, W_v, W_o):
    nc = _get_nc()
    in_maps = make_in_maps(x, rope_freqs, W_q, W_k, W_v, W_o)
    res = run_bass_kernel_spmd(nc, in_maps, core_ids=list(range(NC)))
    out = np.empty((S, D), np.float32)
    for c in range(NC):
        out[c::NC, :] = res.results[c]["y"]
    return out.reshape(1, S, D)
